# revision 34
# baseline (speedup 1.0000x reference)
"""Trainium2 Bass kernel for bidirectional cross-attention.

Problem: nn_CrossAttention (B=8, S1=S2=2048, IN1=256, IN2=768, H=1, D=256).
Sharding: data-parallel over batch; one batch element per NeuronCore (8 cores).
Weights replicated. All matmuls run as float32r (full PE rate at N>=256,
~1.6e-4 rel err measured on HW). fp32r requires even free sizes, hence the
ones-augmented v has 2 padding columns (D+2).

Self-contained: hardcodes shapes; only imports the system concourse package.
"""
import sys

for _p in ("/opt/trn_rl_repo", "/opt/pypackages"):
    if _p not in sys.path:
        sys.path.append(_p)

import numpy as np
import concourse.bass as bass  # noqa: F401
import concourse.mybir as mybir
import concourse.tile as tile
from concourse import bacc
from concourse.bass_utils import run_bass_kernel_spmd
from concourse.masks import make_identity

B = 8
S = 2048          # S1 == S2
IN1 = 256
IN2 = 768
D = 256           # H*D, H=1
NCORES = 8
P = 128
F32 = mybir.dt.float32
F32R = mybir.dt.float32r
EXPF = mybir.ActivationFunctionType.Exp
SCALE = 1.0 / 16.0  # 1/sqrt(D)

NS = S // P       # 16 seq chunks of 128
NQ = S // 512     # 4 q chunks of 512
C1 = IN1 // P     # 2 in-feature chunks for x1
C2 = IN2 // P     # 6 in-feature chunks for x2
DC = D // P       # 2 d chunks

_CACHE = {}


def _build():
    nc = bacc.Bacc(None, target_bir_lowering=False)

    x1_d = nc.dram_tensor("x1", [S, IN1], F32, kind="ExternalInput")
    x2_d = nc.dram_tensor("x2", [S, IN2], F32, kind="ExternalInput")
    wq1_d = nc.dram_tensor("Wq1", [IN1, D], F32, kind="ExternalInput")
    wk1_d = nc.dram_tensor("Wk1", [IN1, D], F32, kind="ExternalInput")
    wv1_d = nc.dram_tensor("Wv1", [IN1, D], F32, kind="ExternalInput")
    wq2_d = nc.dram_tensor("Wq2", [IN2, D], F32, kind="ExternalInput")
    wk2_d = nc.dram_tensor("Wk2", [IN2, D], F32, kind="ExternalInput")
    wv2_d = nc.dram_tensor("Wv2", [IN2, D], F32, kind="ExternalInput")
    wo1_d = nc.dram_tensor("Wo1", [D, IN1], F32, kind="ExternalInput")
    wo2_d = nc.dram_tensor("Wo2", [D, IN2], F32, kind="ExternalInput")
    b1_d = nc.dram_tensor("bo1b", [P, IN1], F32, kind="ExternalInput")
    b2_d = nc.dram_tensor("bo2b", [P, IN2], F32, kind="ExternalInput")
    out1_d = nc.dram_tensor("out1", [S, IN1], F32, kind="ExternalOutput")
    out2_d = nc.dram_tensor("out2", [S, IN2], F32, kind="ExternalOutput")

    def wrearr(d):
        return d[:].rearrange("(c p) o -> p c o", p=P).bitcast(F32R)

    with tile.TileContext(nc) as tc:
        with (
            tc.tile_pool(name="const", bufs=1) as cpool,
            tc.tile_pool(name="qkv", bufs=1) as qkv,
        ):
            ident = cpool.tile([P, P], F32, name="ident")
            make_identity(nc, ident[:])
            ones_f = cpool.tile([P, 1], F32, name="ones")
            nc.vector.memset(ones_f[:], 1.0)
            ones_row_f = cpool.tile([1, P], F32, name="ones_row_f")
            nc.vector.memset(ones_row_f[:], 1.0)
            ones_row = cpool.tile([1, P], F32R, name="ones_row")
            nc.vector.tensor_copy(ones_row[:], ones_row_f[:])

            def transpose_x(xt_for_sc, nchunks, xT, pst):
                for sc in range(NS):
                    xt = xt_for_sc(sc)
                    for ic in range(nchunks):
                        pt = pst.tile([P, P], F32, tag="pst", name="pst")
                        nc.tensor.transpose(
                            pt[:], xt[:, ic * P:(ic + 1) * P], ident[:])
                        nc.vector.tensor_copy(
                            xT[:, ic, sc * P:(sc + 1) * P], pt[:])

            def project(nchunks, xT, wq, wk, wv, qT, kT, va, pp):
                # qT / kT projections: [P, DC, S]
                for dst, w in ((qT, wq), (kT, wk)):
                    for dc in range(DC):
                        for ns in range(NQ):
                            pt = pp.tile([P, 512], F32, tag="pp", name="pp")
                            for ic in range(nchunks):
                                nc.tensor.matmul(
                                    pt[:],
                                    w[:, ic, dc * P:(dc + 1) * P],
                                    xT[:, ic, ns * 512:(ns + 1) * 512],
                                    start=(ic == 0), stop=(ic == nchunks - 1))
                            nc.vector.tensor_copy(
                                dst[:, dc, ns * 512:(ns + 1) * 512], pt[:])
                # v projection (natural [seq, D]) + ones columns
                for sc in range(NS):
                    pt = pp.tile([P, D], F32, tag="pp", name="pp")
                    for ic in range(nchunks):
                        nc.tensor.matmul(
                            pt[:],
                            xT[:, ic, sc * P:(sc + 1) * P],
                            wv[:, ic, :],
                            start=(ic == 0), stop=(ic == nchunks - 1))
                    nc.vector.tensor_copy(va[:, sc, 0:D], pt[:])
                    nc.vector.tensor_copy(va[:, sc, D:D + 1], ones_f[:])
                    nc.vector.tensor_copy(va[:, sc, D + 1:D + 2], ones_f[:])

            with (
                tc.tile_pool(name="xload", bufs=3) as xload,
                tc.tile_pool(name="xload1", bufs=8) as xload1,
                tc.tile_pool(name="w1", bufs=1) as w1p,
                tc.tile_pool(name="pst", bufs=4, space="PSUM") as pst,
                tc.tile_pool(name="pp", bufs=4, space="PSUM") as pp,
            ):
                q2T = qkv.tile([P, DC, S], F32R, name="q2T")
                k2T = qkv.tile([P, DC, S], F32R, name="k2T")
                v2a = qkv.tile([P, NS, D + 2], F32R, name="v2a")
                q1T = qkv.tile([P, C1, S], F32R, name="q1T")
                k1T = qkv.tile([P, C1, S], F32R, name="k1T")
                v1a = qkv.tile([P, NS, D + 2], F32R, name="v1a")
                wq1 = w1p.tile([P, C1, D], F32R, name="wq1")
                wk1 = w1p.tile([P, C1, D], F32R, name="wk1")
                wv1 = w1p.tile([P, C1, D], F32R, name="wv1")
                x1_tiles = []
                with (
                    tc.tile_pool(name="w2", bufs=1) as w2p,
                    tc.tile_pool(name="x2T", bufs=1) as x2tp,
                ):
                    # x2: streaming loads + transposes
                    x2T = x2tp.tile([P, C2, S], F32R, name="x2T")

                    def load_x2(sc):
                        xt = xload.tile([P, IN2], F32, tag="xt", name="xt")
                        nc.sync.dma_start(
                            xt[:], x2_d[sc * P:(sc + 1) * P, :])
                        return xt
                    wq2 = w2p.tile([P, C2, D], F32R, name="wq2")
                    wk2 = w2p.tile([P, C2, D], F32R, name="wk2")
                    wv2 = w2p.tile([P, C2, D], F32R, name="wv2")

                    transpose_x(load_x2, C2, x2T, pst)
                    # HWDGE FIFO order: all 16 x2 chunks, then x1 chunks,
                    # then weights -- everything lands before it is needed
                    for sc in range(NS):
                        t = xload1.tile([P, IN1], F32, tag="xt1", name="xt1")
                        nc.sync.dma_start(
                            t[:], x1_d[sc * P:(sc + 1) * P, :])
                        x1_tiles.append(t)
                    nc.sync.dma_start(wq2[:], wrearr(wq2_d))
                    nc.sync.dma_start(wk2[:], wrearr(wk2_d))
                    nc.sync.dma_start(wv2[:], wrearr(wv2_d))
                    nc.sync.dma_start(wq1[:], wrearr(wq1_d))
                    nc.sync.dma_start(wk1[:], wrearr(wk1_d))
                    nc.sync.dma_start(wv1[:], wrearr(wv1_d))
                    project(C2, x2T, wq2, wk2, wv2, q2T, k2T, v2a, pp)
                with tc.tile_pool(name="x1T", bufs=1) as x1tp:
                    x1T = x1tp.tile([P, C1, S], F32R, name="x1T")
                    transpose_x(lambda sc: x1_tiles[sc], C1, x1T, pst)
                    project(C1, x1T, wq1, wk1, wv1, q1T, k1T, v1a, pp)

            # ---- both attentions, software-pipelined flat loop ----
            with tc.tile_pool(name="oT", bufs=1) as otp:
                o1T = otp.tile([P, DC, S], F32R, name="o1T")
                o2T = otp.tile([P, DC, S], F32R, name="o2T")
                # out-proj weights: DMA early so outproj never stalls
                with tc.tile_pool(name="wo", bufs=1) as wop:
                    wo1 = wop.tile([P, DC, IN1], F32R, name="wo1")
                    wo2 = wop.tile([P, DC, IN2], F32R, name="wo2")
                    b1t = wop.tile([P, IN1], F32, name="b1t")
                    b2t = wop.tile([P, IN2], F32, name="b2t")
                    nc.sync.dma_start(wo1[:], wrearr(wo1_d))
                    nc.sync.dma_start(wo2[:], wrearr(wo2_d))
                    nc.sync.dma_start(b1t[:], b1_d[:])
                    nc.sync.dma_start(b2t[:], b2_d[:])

                    cfgs = [(q1T, k2T, v2a, o1T), (q2T, k1T, v1a, o2T)]
                    with (
                        tc.tile_pool(name="outs", bufs=8) as outs,
                        tc.tile_pool(name="exp", bufs=8) as expp,
                        tc.tile_pool(name="on", bufs=6) as onp,
                        tc.tile_pool(name="rd", bufs=8) as rdp,
                        tc.tile_pool(name="sT", bufs=2, space="PSUM") as sTp,
                        tc.tile_pool(name="oag", bufs=1, space="PSUM") as oagp,
                        tc.tile_pool(name="pst2", bufs=2, space="PSUM") as ps2,
                    ):
                        def emit_qk_exp(ci, qc, kc):
                            qT, kT, va, oT = cfgs[ci]
                            sT = sTp.tile([P, 512], F32, tag="sT", name="sT")
                            for dc in range(DC):
                                nc.tensor.matmul(
                                    sT[:],
                                    kT[:, dc, kc * P:(kc + 1) * P],
                                    qT[:, dc, qc * 512:(qc + 1) * 512],
                                    start=(dc == 0), stop=(dc == DC - 1))
                            ex = expp.tile([P, 512], F32R, tag="ex", name="ex")
                            nc.scalar.activation(
                                ex[:], sT[:], EXPF, scale=SCALE)
                            return ex

                        steps = [(ci, qc, kc)
                                 for ci in range(2)
                                 for qc in range(NQ)
                                 for kc in range(NS)]
                        DEPTH = 3
                        exq = [emit_qk_exp(*steps[j])
                               for j in range(DEPTH)]
                        oag = None
                        for i, (ci, qc, kc) in enumerate(steps):
                            ex = exq.pop(0)
                            if i + DEPTH < len(steps):
                                exq.append(emit_qk_exp(*steps[i + DEPTH]))
                            if kc == 0:
                                oag = [oagp.tile([P, D + 2], F32,
                                                 tag=f"oag{qs}",
                                                 name=f"oag{qs}")
                                       for qs in range(4)]
                            va = cfgs[ci][2]
                            for qs in range(4):
                                nc.tensor.matmul(
                                    oag[qs][:],
                                    ex[:, qs * P:(qs + 1) * P],
                                    va[:, kc, :],
                                    start=(kc == 0), stop=(kc == NS - 1))
                            if kc == NS - 1:
                                oT = cfgs[ci][3]
                                for qs in range(4):
                                    rd = rdp.tile([P, 1], F32, tag="rd",
                                                  name="rd")
                                    nc.vector.reciprocal(
                                        rd[:], oag[qs][:, D:D + 1])
                                    on = onp.tile([P, D], F32, tag="on",
                                                  name="on")
                                    nc.vector.tensor_scalar_mul(
                                        on[:], oag[qs][:, 0:D], rd[:])
                                    for dc in range(DC):
                                        pt = ps2.tile([P, P], F32, tag="pst2",
                                                      name="pst2")
                                        nc.tensor.transpose(
                                            pt[:], on[:, dc * P:(dc + 1) * P],
                                            ident[:])
                                        base = qc * 512 + qs * P
                                        nc.vector.tensor_copy(
                                            oT[:, dc, base:base + P], pt[:])
                                # output-project the 4 finished seq chunks
                                # (psum slots shared with the transposes;
                                # copies on DVE only -- ACT keeps its exp
                                # table; bias folded as rank-1 ones matmul)
                                for qs in range(4):
                                    sc = qc * 4 + qs
                                    s0, s1 = sc * P, (sc + 1) * P
                                    if ci == 0:
                                        pt = ps2.tile([P, IN1], F32,
                                                      tag="pst2", name="po")
                                        for dc in range(DC):
                                            nc.tensor.matmul(
                                                pt[:], o1T[:, dc, s0:s1],
                                                wo1[:, dc, :],
                                                start=(dc == 0),
                                                stop=(dc == DC - 1))
                                        ot = outs.tile([P, IN1], F32,
                                                       tag="ot1", name="ot1")
                                        nc.vector.tensor_add(
                                            ot[:], pt[:], b1t[:])
                                        nc.sync.dma_start(
                                            out1_d[s0:s1, :], ot[:])
                                    else:
                                        H2 = IN2 // 2
                                        for h in range(2):
                                            pt = ps2.tile([P, H2], F32,
                                                          tag="pst2",
                                                          name="po2")
                                            for dc in range(DC):
                                                nc.tensor.matmul(
                                                    pt[:],
                                                    o2T[:, dc, s0:s1],
                                                    wo2[:, dc,
                                                        h * H2:(h + 1) * H2],
                                                    start=(dc == 0),
                                                    stop=(dc == DC - 1))
                                            ot = outs.tile(
                                                [P, H2], F32, tag="ot2",
                                                name="ot2")
                                            nc.vector.tensor_add(
                                                ot[:], pt[:],
                                                b2t[:, h * H2:(h + 1) * H2])
                                            nc.sync.dma_start(
                                                out2_d[s0:s1,
                                                       h * H2:(h + 1) * H2],
                                                ot[:])

    nc.finalize()
    return nc


def _get_nc():
    if "nc" not in _CACHE:
        _CACHE["nc"] = _build()
    return _CACHE["nc"]


def kernel(x1, x2, Wq1, Wk1, Wv1, Wq2, Wk2, Wv2, Wo1, bo1, Wo2, bo2,
           _trace=False):
    nc = _get_nc()
    b1b = np.ascontiguousarray(
        np.broadcast_to(np.asarray(bo1, np.float32), (P, IN1)))
    b2b = np.ascontiguousarray(
        np.broadcast_to(np.asarray(bo2, np.float32), (P, IN2)))
    shared = {
        "Wq1": np.asarray(Wq1, np.float32), "Wk1": np.asarray(Wk1, np.float32),
        "Wv1": np.asarray(Wv1, np.float32), "Wq2": np.asarray(Wq2, np.float32),
        "Wk2": np.asarray(Wk2, np.float32), "Wv2": np.asarray(Wv2, np.float32),
        "Wo1": np.asarray(Wo1, np.float32), "Wo2": np.asarray(Wo2, np.float32),
        "bo1b": b1b, "bo2b": b2b,
    }
    in_maps = []
    for b in range(B):
        m = dict(shared)
        m["x1"] = np.ascontiguousarray(np.asarray(x1[b], np.float32))
        m["x2"] = np.ascontiguousarray(np.asarray(x2[b], np.float32))
        in_maps.append(m)
    res = run_bass_kernel_spmd(
        nc, in_maps, core_ids=list(range(NCORES)), trace=_trace)
    out1 = np.stack([res.results[b]["out1"] for b in range(B)])
    out2 = np.stack([res.results[b]["out2"] for b in range(B)])
    if _trace:
        _CACHE["last_result"] = res
    return out1, out2
